# revision 3
# baseline (speedup 1.0000x reference)
"""Trainium2 Bass kernel for nn_ConfidenceFilter (3-layer MLP with per-sample
early exit on softmax confidence).

Reference computation (B=8192, D=H=2048, C=1000):
    h1 = relu(x@W1+b1); p1 = h1@H1w+H1b; c1 = max softmax(p1) > 0.01
    h2 = relu(h1@W2+b2); p2 = h2@H2w+H2b; c2 = max softmax(p2) > 0.01
    h3 = relu(h2@W3+b3); p3 = h3@Fw+Fb
    out = where(c1, p1, where(c2, p2, p3))

Sharding: pure data parallel over 8 NeuronCores (1024 batch rows each).
All stage weights replicated. Within a core the batch is processed in two
halves of 512 rows so activations fit SBUF.

Layout: activations are kept transposed in SBUF (hT = [feature_part, batch])
so every backbone layer chains stationary=W-chunk / moving=hT. Heads flip:
stationary=hT-chunk, moving=Hw slices, giving logits as [batch_part, class]
so the confidence reduction is a free-dim reduce.

Precision: the c1 mask has samples as close as 2.7e-5 (relative) to the
threshold, so W1/H1w matmuls run in true fp32 (4-pass, ~1.7e-7). The c2 mask
margin is 4.6e-2 and p2/p3 only contribute output values, so W2/H2w/W3/Fw run
single-pass float32r (RNE-11-mantissa operands, fp32 accumulate, ~3e-4).

max-softmax-prob > t  <=>  sum_j exp(p_j - max) < 1/t, so only a row max and
an exp-accumulate (ScalarE accum_out) are needed per logit row, no full
softmax.
"""

import numpy as np
from contextlib import ExitStack

import concourse.bass as bass
import concourse.mybir as mybir
import concourse.tile as tile
from concourse import bacc
from concourse.bass_utils import run_bass_kernel_spmd

f32 = mybir.dt.float32
f32r = mybir.dt.float32r
AF = mybir.ActivationFunctionType
OP = mybir.AluOpType
AX = mybir.AxisListType

N_CORES = 8
THRESH_INV = 100.0  # 1/0.01: confident iff sum(exp(p - max)) < 100


def build(D=2048, H=2048, C=1000, BC=1024, HALF=512):
    """Build the per-core Bass program. All dims divisible by 128; C split
    into 2 windows of C//2 (<=512) columns."""
    KC = D // 128          # k chunks for layer 1
    NC = H // 128          # hidden chunks (also k chunks for layers 2/3)
    MC = HALF // 128       # batch chunks of 128 within a half
    NHALF = BC // HALF     # number of halves
    CW = C // 2            # class window (<=512)
    assert C % 2 == 0 and CW <= 512

    nc = bacc.Bacc("TRN2", target_bir_lowering=False, debug=False,
                   num_devices=N_CORES)

    def din(name, shape):
        return nc.dram_tensor(name, shape, f32, kind="ExternalInput").ap()

    xT = din("xT", [D, BC])
    W = [din("W1", [D, H]), din("W2", [H, H]), din("W3", [H, H])]
    bvec = [din("b1", [H]), din("b2", [H]), din("b3", [H])]
    Hw = [din("H1w", [H, C]), din("H2w", [H, C]), din("Fw", [H, C])]
    Hb = [din("H1b", [1, C]), din("H2b", [1, C]), din("Fb", [1, C])]
    out = nc.dram_tensor("out", [BC, C], f32, kind="ExternalOutput").ap()

    with tile.TileContext(nc) as tc, ExitStack() as ctx:
        pool = lambda name, bufs, **kw: ctx.enter_context(
            tc.tile_pool(name=name, bufs=bufs, **kw))

        sb_big = pool("big", 1)       # xt(h) then h3(h), rotating: [128,KC,HALF]
        sb_h1a = pool("h1a", 1)       # h1 fp32 (head-1 stationary)
        sb_h1b = pool("h1b", 1)       # h1 f32r (layer-2 moving)
        sb_h2 = pool("h2", 1)         # h2 f32r
        sb_acc = pool("acc", 1)       # blend state [128,MC,C] fp32
        sb_wst = pool("wst", 3)       # backbone weight col-blocks [128,KC,128]
        sb_hwst = pool("hwst", 3)     # head weight slices [128,CW]
        sb_hbc = pool("hbc", 2)       # head bias broadcast [128,CW]
        sb_bias = pool("bias", 3)     # backbone bias [128,NC]
        sb_esc = pool("esc", 2)       # exp scratch [128,CW] (write-only sink)
        sb_stage = pool("stage", 3)   # final out staging [128,CW]
        sb_dt = pool("dt", 3)         # blend diff [128,CW]
        sb_stat = pool("stat", 32)    # [128,1] stats
        sb_mask = pool("mask", 24)    # c1/f2 masks [128,1]
        sb_k = pool("k", 4)           # constants / junk

        ps_bb = pool("psbb", 2, space="PSUM")   # [128,HALF]
        ps_hd = pool("pshd", 5, space="PSUM")   # [128,CW]
        ps_j = pool("psj", 1, space="PSUM")     # [1,1] join target

        # ---- preamble ----
        zjoin = sb_k.tile([1, 1], f32, tag="zjoin")
        nc.vector.memset(zjoin[:], 0.0)
        jps = ps_j.tile([1, 64], f32, tag="jps")
        nc.tensor.matmul(jps[0:1, 0:1], lhsT=zjoin[:], rhs=zjoin[:],
                         start=True, stop=True)

        def pe_join(ap_f32):
            """Absorb one fresh semaphore wait on PE via a tiny matmul so the
            following real matmuls stay within walrus' 1-wait/matmul limit."""
            nfree = ap_f32.free_size()
            nc.tensor.matmul(jps[0:1, 0:nfree], lhsT=zjoin[:], rhs=ap_f32,
                             start=True, stop=True)

        awarm = sb_k.tile([1, 1], f32, tag="awarm")
        nc.scalar.activation(awarm[:], zjoin[:], AF.Exp)  # load ACT exp tables

        ajunk = sb_k.tile([1, 1], f32, tag="ajunk")
        vjunk = sb_k.tile([1, 1], f32, tag="vjunk")

        # backbone bias tiles [128, NC] (bias for hidden unit n*128+p at [p,n])
        btiles = []
        for li in range(3):
            bt = sb_bias.tile([128, NC], f32, tag="bias")
            nc.sync.dma_start(bt[:], bvec[li].rearrange("(n p) -> p n", p=128))
            nc.scalar.copy(ajunk[:], bt[0:1, 0:1])  # ACT join on the DMA
            btiles.append(bt)

        h_prev = None  # [128, KC|NC, HALF] tile holding previous activations

        def backbone(li, h, rhs_tile, rhs_chunks, fp32_mode, out_hi, out_32):
            """hT_out[n,:] = relu(W[li][:,n-chunk].T @ rhs + b). Writes f32r
            copy to out_hi and (if not None) fp32 copy to out_32."""
            Wd = W[li] if fp32_mode else W[li].bitcast(f32r)
            wdt = f32 if fp32_mode else f32r
            # phase join: one tiny matmul reading one element of every rhs
            # chunk -> PE observes the newest producer tick of the rhs tile
            pe_join(rhs_tile[0:1, :, 0:1].bitcast(f32))
            for n in range(NC):
                wblk = sb_wst.tile([128, rhs_chunks, 128], wdt, tag="wst")
                nc.sync.dma_start(
                    wblk[:], Wd[:, n * 128:(n + 1) * 128]
                    .rearrange("(kc p) m -> p kc m", p=128))
                pe_join(wblk[0:1, 0, 0:1].bitcast(f32))
                ps = ps_bb.tile([128, HALF], f32, tag="psbb")
                for kc in range(rhs_chunks):
                    nc.tensor.matmul(ps[:], lhsT=wblk[:, kc, :],
                                     rhs=rhs_tile[:, kc, :],
                                     start=(kc == 0), stop=(kc == rhs_chunks - 1))
                bias_ap = btiles[li][:, n:n + 1]
                if out_32 is not None:
                    nc.scalar.activation(out_32[:, n, :], ps[:], AF.Relu,
                                         bias=bias_ap)
                    nc.scalar.copy(out_hi[:, n, :], out_32[:, n, :])
                else:
                    nc.scalar.activation(out_hi[:, n, :], ps[:], AF.Relu,
                                         bias=bias_ap)

        def head(hi, h, stat_tile, fp32_mode, c1_masks, f2_masks, acc_t,
                 final):
            """Head hi over stationary stat_tile ([128,NC,HALF] chunks).
            hi: 0 -> write p1 into acc + compute c1
            hi: 1 -> blend p2 via c1, compute f2 = c1|c2
            hi: 2 -> blend p3 via f2, stage+DMA final output rows."""
            Wd = Hw[hi] if fp32_mode else Hw[hi].bitcast(f32r)
            wdt = f32 if fp32_mode else f32r
            pe_join(stat_tile[0:1, :, 0:1].bitcast(f32))
            Ms, Ss = {}, {}
            for nw in range(2):
                csl = slice(nw * CW, (nw + 1) * CW)
                # head bias broadcast tile for this (head, nw)
                hbc = sb_hbc.tile([128, CW], f32, tag="hbc")
                nc.sync.dma_start(hbc[:], Hb[hi][0:1, csl].broadcast_to((128, CW)))
                nc.vector.tensor_copy(vjunk[:], hbc[0:1, 0:1])  # DVE join
                phs = []
                for kc in range(NC):
                    hw = sb_hwst.tile([128, CW], wdt, tag="hwst")
                    nc.sync.dma_start(hw[:], Wd[kc * 128:(kc + 1) * 128, csl])
                    pe_join(hw[0:1, 0:1].bitcast(f32))
                    for mc in range(MC):
                        if kc == 0:
                            ph_new = ps_hd.tile([128, CW], f32, tag="pshd")
                            phs.append(ph_new)
                        nc.tensor.matmul(
                            phs[mc][:],
                            lhsT=stat_tile[:, kc, mc * 128:(mc + 1) * 128],
                            rhs=hw[:], start=(kc == 0), stop=(kc == NC - 1))
                for mc in range(MC):
                    ph = phs[mc]
                    # bias (varies along free dim) via DVE add into PSUM
                    nc.vector.tensor_tensor(ph[:], ph[:], hbc[:], op=OP.add)
                    if hi < 2:
                        # stats: local max + exp-sum of this class window
                        M = sb_stat.tile([128, 1], f32, tag="stat")
                        nc.vector.tensor_reduce(M[:], ph[:], axis=AX.X, op=OP.max)
                        negm = sb_stat.tile([128, 1], f32, tag="stat")
                        nc.vector.tensor_scalar(negm[:], M[:], -1.0, None,
                                                op0=OP.mult)
                        esc = sb_esc.tile([128, CW], f32, tag="esc")
                        S = sb_stat.tile([128, 1], f32, tag="stat")
                        nc.scalar.activation(esc[:], ph[:], AF.Exp,
                                             bias=negm[:], accum_out=S[:])
                        Ms[(mc, nw)] = M
                        Ss[(mc, nw)] = S
                    if hi == 0:
                        nc.scalar.copy(acc_t[:, mc, csl], ph[:])
                    else:
                        sel = c1_masks[mc] if hi == 1 else f2_masks[mc]
                        dt = sb_dt.tile([128, CW], f32, tag="dt")
                        nc.vector.tensor_tensor(dt[:], acc_t[:, mc, csl], ph[:],
                                                op=OP.subtract)
                        dst = acc_t[:, mc, csl]
                        if hi == 2:
                            stg = sb_stage.tile([128, CW], f32, tag="stage")
                            dst = stg[:]
                        nc.vector.scalar_tensor_tensor(
                            dst, in0=dt[:], scalar=sel[:], in1=ph[:],
                            op0=OP.mult, op1=OP.add)
                        if hi == 2:
                            r0 = h * HALF + mc * 128
                            nc.gpsimd.dma_start(out[r0:r0 + 128, csl], stg[:])
            if hi == 2:
                return
            # combine the two class windows: s = s0*e^(M0-M) + s1*e^(M1-M)
            for mc in range(MC):
                M0, M1 = Ms[(mc, 0)], Ms[(mc, 1)]
                S0, S1 = Ss[(mc, 0)], Ss[(mc, 1)]
                M = sb_stat.tile([128, 1], f32, tag="stat")
                nc.vector.tensor_tensor(M[:], M0[:], M1[:], op=OP.max)
                s_tot = sb_stat.tile([128, 1], f32, tag="stat")
                first = True
                for Mi, Si in ((M0, S0), (M1, S1)):
                    d = sb_stat.tile([128, 1], f32, tag="stat")
                    nc.vector.tensor_tensor(d[:], Mi[:], M[:], op=OP.subtract)
                    e = sb_stat.tile([128, 1], f32, tag="stat")
                    nc.scalar.activation(e[:], d[:], AF.Exp)
                    t = sb_stat.tile([128, 1], f32, tag="stat")
                    nc.vector.tensor_tensor(t[:], Si[:], e[:], op=OP.mult)
                    if first:
                        nc.vector.tensor_copy(s_tot[:], t[:])
                        first = False
                    else:
                        nc.vector.tensor_tensor(s_tot[:], s_tot[:], t[:],
                                                op=OP.add)
                c = sb_mask.tile([128, 1], f32, tag=f"c{hi}")
                nc.vector.tensor_scalar(c[:], s_tot[:], THRESH_INV, None,
                                        op0=OP.is_lt)
                if hi == 0:
                    c1_masks[mc] = c
                else:
                    f2 = sb_mask.tile([128, 1], f32, tag="f2")
                    nc.vector.tensor_tensor(f2[:], c1_masks[mc][:], c[:],
                                            op=OP.max)
                    f2_masks[mc] = f2

        for h in range(NHALF):
            msl = slice(h * HALF, (h + 1) * HALF)
            # load xT half (fp32 for the fp32 layer-1 matmul)
            xt = sb_big.tile([128, KC, HALF], f32, tag="big")
            for kc in range(KC):
                nc.sync.dma_start(
                    xt[:, kc, :], xT[kc * 128:(kc + 1) * 128, msl])

            h1_32 = sb_h1a.tile([128, NC, HALF], f32, tag="h1a")
            h1_hi = sb_h1b.tile([128, NC, HALF], f32r, tag="h1b")
            backbone(0, h, xt, KC, True, h1_hi, h1_32)

            acc_t = sb_acc.tile([128, MC, C], f32, tag="acc")
            c1_masks, f2_masks = {}, {}
            head(0, h, h1_32, True, c1_masks, f2_masks, acc_t, False)

            h2_t = sb_h2.tile([128, NC, HALF], f32r, tag="h2")
            backbone(1, h, h1_hi, NC, False, h2_t, None)
            head(1, h, h2_t, False, c1_masks, f2_masks, acc_t, False)

            h3_t = sb_big.tile([128, NC, HALF], f32r, tag="big")
            backbone(2, h, h2_t, NC, False, h3_t, None)
            head(2, h, h3_t, False, c1_masks, f2_masks, acc_t, True)

    nc.compile()
    return nc


_cached = {}


def _get_nc():
    if "nc" not in _cached:
        _cached["nc"] = build()
    return _cached["nc"]


def kernel(x, W1, b1, W2, b2, W3, b3, H1w, H1b, H2w, H2b, Fw, Fb,
           _trace=False):
    x = np.ascontiguousarray(np.asarray(x, dtype=np.float32))
    B = x.shape[0]
    BC = B // N_CORES
    C = H1w.shape[1]
    f = lambda a: np.ascontiguousarray(np.asarray(a, dtype=np.float32))
    common = {
        "W1": f(W1), "W2": f(W2), "W3": f(W3),
        "b1": f(b1), "b2": f(b2), "b3": f(b3),
        "H1w": f(H1w), "H2w": f(H2w), "Fw": f(Fw),
        "H1b": f(H1b).reshape(1, C), "H2b": f(H2b).reshape(1, C),
        "Fb": f(Fb).reshape(1, C),
    }
    in_maps = []
    for c in range(N_CORES):
        xTc = np.ascontiguousarray(x[c * BC:(c + 1) * BC].T)
        in_maps.append({"xT": xTc, **common})
    nc = _get_nc()
    res = run_bass_kernel_spmd(nc, in_maps, core_ids=list(range(N_CORES)),
                               trace=_trace)
    kernel._last_exec_time_ns = res.exec_time_ns
    return np.concatenate([res.results[c]["out"] for c in range(N_CORES)],
                          axis=0)
